# revision 18
# baseline (speedup 1.0000x reference)
"""Causal attention layer on 8 TRN2 NeuronCores, data-parallel over batch.

Per-core problem (batch element n = core id):
    q = query @ Wq.T ; k = key @ Wk.T              (f32r matmuls)
    scores[s,t] = q[s]·k[t]  for t <= s            (f32r)
    attn = softmax(32 * scores)  (the +1 additive mask cancels in softmax;
                                  -inf masking == skipping t > s)
    ctx[s,i] = sum_t attn[s,t] * value[t,i]        (bf16)
    out[s,:] = (ctx @ Wv.T) / rowsum               (bf16, normalization folded)

v2 layout: all bf16 transposes (attnT, ctx, WvT) ride the DMA XBAR
transpose engine (serialized on the SP HWDGE queue — concurrent xbar
transposes on both queues corrupt each other); only the f32 q/k strip
transposes stay on the PE.  The M = G @ key^T projection is interleaved
into the early strip iterations to fill the PE bubbles the softmax
chain leaves there.
"""
import numpy as np
from contextlib import ExitStack

import concourse.bass as bass
import concourse.tile as tile
from concourse import bacc, mybir
from concourse.bass_utils import run_bass_kernel_spmd
from concourse.masks import make_identity

F32 = mybir.dt.float32
F32R = mybir.dt.float32r
BF16 = mybir.dt.bfloat16

N, S, T, D = 8, 2048, 2048, 1024
P = 128
NSTRIP = S // P          # 16 query strips
TCH = 512                # t-chunk for score matmuls
CW = 512                 # projection chunk width
OC = D // P              # 8 chunks of the projection/feature dim
NCHK = T // CW           # 4 key chunks
SCALE = float(np.sqrt(np.float32(D)))  # 32.0
NEG = -1.0e30

QK_DT = F32R             # q/k projections + scores matmuls

PHASE_MARKS = []


def _mark(nc, label):
    PHASE_MARKS.append((label, nc.next_id()))


def _mm(nc, out, lhsT, rhs, dt, **kw):
    nc.tensor.matmul(out, lhsT.bitcast(dt), rhs.bitcast(dt), **kw)


def build_nc():
    PHASE_MARKS.clear()
    nc = bacc.Bacc("TRN2", target_bir_lowering=False, debug=False,
                   enable_asserts=False)
    _dma_rr = [0]

    def dma(out, in_):
        # startup loads/stores alternate the two HWDGE queues
        _dma_rr[0] ^= 1
        eng = nc.sync if _dma_rr[0] else nc.scalar
        return eng.dma_start(out, in_)

    def xbar(out, in_):
        # ALL xbar transposes on the SP queue (they corrupt if concurrent)
        return nc.sync.dma_start(out, in_, transpose=True)

    q_d = nc.dram_tensor("query", [S, D], F32, kind="ExternalInput")
    k_d = nc.dram_tensor("key", [T, D], F32, kind="ExternalInput")
    v_d = nc.dram_tensor("value", [T, D], F32, kind="ExternalInput")
    wq_d = nc.dram_tensor("Wq", [D, D], F32, kind="ExternalInput")
    wk_d = nc.dram_tensor("Wk", [D, D], F32, kind="ExternalInput")
    wv_d = nc.dram_tensor("Wv", [D, D], F32, kind="ExternalInput")
    out_d = nc.dram_tensor("out", [S, D], F32, kind="ExternalOutput")

    with tile.TileContext(nc) as tc, ExitStack() as ctx:
        const = ctx.enter_context(tc.tile_pool(name="const", bufs=1))
        wt_pool = ctx.enter_context(tc.tile_pool(name="wt", bufs=1))
        wvt_pool = ctx.enter_context(tc.tile_pool(name="wvt", bufs=1))
        kt_pool = ctx.enter_context(tc.tile_pool(name="kt", bufs=1))
        val_pool = ctx.enter_context(tc.tile_pool(name="val", bufs=1))
        stage = ctx.enter_context(tc.tile_pool(name="stage", bufs=2))
        qst_pool = ctx.enter_context(tc.tile_pool(name="qst", bufs=1))
        int_pool = ctx.enter_context(tc.tile_pool(name="inT", bufs=1))
        qts_pool = ctx.enter_context(tc.tile_pool(name="qts", bufs=1))
        sc_pool = ctx.enter_context(tc.tile_pool(name="scores", bufs=2))
        exp_pool = ctx.enter_context(tc.tile_pool(name="exp", bufs=1))
        at_pool = ctx.enter_context(tc.tile_pool(name="attnT", bufs=1))
        ctxn_pool = ctx.enter_context(tc.tile_pool(name="ctxn", bufs=1))
        ctxsb_pool = ctx.enter_context(tc.tile_pool(name="ctxsb", bufs=1))
        ob_pool = ctx.enter_context(tc.tile_pool(name="outb", bufs=1))
        st_pool = ctx.enter_context(tc.tile_pool(name="stats", bufs=32))
        sc_ps = ctx.enter_context(tc.tile_pool(name="scps", bufs=2, space="PSUM"))
        tp_ps = ctx.enter_context(tc.tile_pool(name="tpps", bufs=2, space="PSUM"))
        av_ps = ctx.enter_context(tc.tile_pool(name="avps", bufs=2, space="PSUM"))

        ident = const.tile([P, P], F32)
        make_identity(nc, ident)
        # mask128[p, x] = 0 if x <= p else -1e30 (universal diag boundary)
        mask128 = const.tile([P, P], BF16)
        nc.gpsimd.memset(mask128[:], 0.0)
        nc.gpsimd.affine_select(
            out=mask128[:], in_=mask128[:],
            compare_op=mybir.AluOpType.is_ge, fill=NEG,
            base=0, channel_multiplier=1, pattern=[[-1, P]])

        _mm_ps_rr = [0]

        def mm_ps_tile():
            # rotate [P, TCH] psum tiles across the two 2-buf pools
            _mm_ps_rr[0] ^= 1
            pool = sc_ps if _mm_ps_rr[0] else tp_ps
            return pool.tile([P, TCH], F32, name="mmps",
                             tag="sc" if _mm_ps_rr[0] else "tp")

        # ---- phase G: gT[j, i] = (Wq^T @ Wk)^T = Wk^T @ Wq ----
        _mark(nc, 'phaseG')
        # prefetch key strips 0-1 for kT chunk 0 through the (still idle)
        # scores pool so the stage pool stays free for the weight stream
        xt0 = int_pool.tile([P, OC, CW], QK_DT, name="inT", tag="inT")
        k0stg = []
        for sl in range(2):
            stg = sc_pool.tile([P, D], F32, name="scores")
            nc.gpsimd.dma_start(stg[:], k_d.ap()[sl * P:(sl + 1) * P, :])
            k0stg.append(stg)

        def kT0_group(first):
            sls = (0, 1) if first else (2, 3)
            for sl in sls:
                stg = k0stg[sl]
                for g in range(0, OC, 4):
                    ps = tp_ps.tile([P, TCH], F32, name="mmps", tag="tp")
                    for c in range(4):
                        nc.tensor.transpose(
                            ps[:, c * P:(c + 1) * P],
                            stg[:, (g + c) * P:(g + c + 1) * P], ident[:])
                    nc.vector.tensor_copy(
                        xt0[:, g:g + 4, sl * P:(sl + 1) * P],
                        ps.rearrange("p (c s) -> p c s", c=4))
            if first:
                for sl in (2, 3):
                    stg = sc_pool.tile([P, D], F32, name="scores")
                    nc.gpsimd.dma_start(
                        stg[:], k_d.ap()[sl * P:(sl + 1) * P, :])
                    k0stg.append(stg)

        wqwk = kt_pool.tile([P, 16, D], QK_DT, name="wqwk", tag="kt")
        for r in range(OC):
            for w_dram, slot in ((wq_d, r), (wk_d, OC + r)):
                for h in range(2):
                    stg = stage.tile([P, TCH], F32, name="stgh", tag="stage")
                    dma(stg[:], w_dram.ap()[r * P:(r + 1) * P,
                                            h * TCH:(h + 1) * TCH])
                    nc.vector.tensor_copy(
                        wqwk[:, slot, h * TCH:(h + 1) * TCH], stg[:])
        gT = wt_pool.tile([P, OC, D], QK_DT, name="wT", tag="wT")
        # o-strip-streamed accumulation: 8 psum banks live at once so the
        # per-oc matmul work covers the per-oc DMA
        for half in range(2):
            grp = []
            for ih, jcg in ((half, 0), (1 - half, 1)):
                if jcg == 0:
                    accs = [mm_ps_tile()[:] for _ in range(4)]
                else:
                    acc_b = av_ps.tile([P, 2, TCH], F32, name="avps",
                                       tag="avps")
                    acc_c = av_ps.tile([P, 2, TCH], F32, name="avps",
                                       tag="avps")
                    accs = [acc_b[:, 0, :], acc_b[:, 1, :],
                            acc_c[:, 0, :], acc_c[:, 1, :]]
                grp.append((ih, jcg, accs))
            for oc in range(OC):
                for ih, jcg, accs in grp:
                    for j in range(4):
                        jc = jcg * 4 + j
                        _mm(nc, accs[j],
                            wqwk[:, OC + oc, jc * P:(jc + 1) * P],
                            wqwk[:, oc, ih * TCH:(ih + 1) * TCH], QK_DT,
                            start=(oc == 0), stop=(oc == OC - 1))
            for ih, jcg, accs in grp:
                for j in range(4):
                    jc = jcg * 4 + j
                    nc.vector.tensor_copy(
                        gT[:, jc, ih * TCH:(ih + 1) * TCH], accs[j])
            kT0_group(first=(half == 0))

        # ---- WvT (bf16) via xbar: wvT[i_loc, ic, j] = Wv[j, ic*128+i_loc]
        _mark(nc, 'phaseWv')
        wvT = wvt_pool.tile([P, OC, D], BF16, name="wvT")
        for js in range(OC):
            stg = stage.tile([P, D], F32, name="stage", tag="stage")
            dma(stg[:], wv_d.ap()[js * P:(js + 1) * P, :])
            wvb = ctxn_pool.tile([P, D], BF16, name="ctxn", tag="ctxn")
            nc.scalar.activation(wvb[:], stg[:],
                                 mybir.ActivationFunctionType.Copy)
            xbar(wvT[:, :, js * P:(js + 1) * P], wvb[:])

        def in_transpose_chunk(x_dram, c0):
            """x[c0:c0+CW, :D] -> xT tile [128(i_loc), OC, CW] (PE)."""
            xt = int_pool.tile([P, OC, CW], QK_DT, name="inT", tag="inT")
            for sl in range(CW // P):
                stg = stage.tile([P, D], F32, name="stage", tag="stage")
                nc.gpsimd.dma_start(
                    stg[:], x_dram.ap()[c0 + sl * P:c0 + (sl + 1) * P, :])
                for g in range(0, OC, 4):
                    ps = tp_ps.tile([P, TCH], F32, name="mmps", tag="tp")
                    for c in range(4):
                        nc.tensor.transpose(
                            ps[:, c * P:(c + 1) * P],
                            stg[:, (g + c) * P:(g + c + 1) * P], ident[:])
                    nc.vector.tensor_copy(
                        xt[:, g:g + 4, sl * P:(sl + 1) * P],
                        ps.rearrange("p (c s) -> p c s", c=4))
            return xt

        # kt written chunk-by-chunk; chunks 1-3 are emitted inside the
        # strip loop so the M matmuls fill early-strip PE bubbles
        kt = kt_pool.tile([P, OC, T], QK_DT, name="kt", tag="kt")

        def m_chunk(sc, kT_in=None):
            """M[:, sc*CW:(sc+1)*CW] = G @ key[sc-chunk]^T."""
            _mark(nc, f'M{sc}')
            if kT_in is None:
                kT_in = in_transpose_chunk(k_d, sc * CW)
            for ic in range(OC):
                ps = mm_ps_tile()
                for jc in range(OC):
                    _mm(nc, ps[:, :CW], gT[:, jc, ic * P:(ic + 1) * P],
                        kT_in[:, jc, :], QK_DT,
                        start=(jc == 0), stop=(jc == OC - 1))
                # ACT copy keeps the DVE queue free for the softmax chain
                nc.scalar.activation(
                    kt[:, ic, sc * CW:(sc + 1) * CW], ps[:, :CW],
                    mybir.ActivationFunctionType.Copy)

        m_chunk(0, kT_in=xt0)

        _mark(nc, 'phaseV')
        val = val_pool.tile([P, T // P, D], BF16)
        state = {}

        def loads(si):
            """value blocks 2si,2si+1 + query strip si (SWDGE, prefetch).
            Value casts ride gpsimd so the DVE queue stays clear for the
            softmax chain."""
            for tb in (2 * si, 2 * si + 1):
                if tb >= T // P:
                    continue
                vstg = stage.tile([P, D], F32, name="stage", tag="stage")
                nc.gpsimd.dma_start(vstg[:],
                                    v_d.ap()[tb * P:(tb + 1) * P, :])
                nc.gpsimd.tensor_copy(val[:, tb, :], vstg[:])
            stg = qst_pool.tile([P, D], F32, name="qst")
            nc.scalar.dma_start(stg[:], q_d.ap()[si * P:(si + 1) * P, :])
            state[(si, 'q')] = stg

        def pass_a(si):
            """scores chunks + row max for strip si (PE + DVE)."""
            _mark(nc, f'strip{si}')
            s0 = si * P
            nch = (s0 + P + TCH - 1) // TCH   # t-chunks covering [0, s0+128)
            stg = state.pop((si, 'q'))
            qts = qts_pool.tile([P, OC, P], QK_DT, name="qts")
            for g in range(0, OC, 4):
                ps = tp_ps.tile([P, TCH], F32, name="mmps", tag="tp")
                for c in range(4):
                    nc.tensor.transpose(
                        ps[:, c * P:(c + 1) * P],
                        stg[:, (g + c) * P:(g + c + 1) * P], ident[:])
                nc.vector.tensor_copy(
                    qts[:, g:g + 4, :],
                    ps.rearrange("p (c s) -> p c s", c=4))

            scores = sc_pool.tile([P, S], F32, name="scores")
            # diag chunk first so its (mask + max) tail overlaps the other
            # chunks' matmuls (f32r needs moving dim >= 256)
            dw = max(2 * P, (si % 4 + 1) * P)
            cmaxes = []
            for c in [nch - 1] + list(range(nch - 1)):
                cw = dw if c == nch - 1 else TCH
                ps = sc_ps.tile([P, TCH], F32, name="mmps", tag="sc")
                for oc in range(OC):
                    _mm(nc, ps[:, :cw], qts[:, oc, :],
                        kt[:, oc, c * TCH:c * TCH + cw], QK_DT,
                        start=(oc == 0), stop=(oc == OC - 1))
                dst = scores[:, c * TCH:c * TCH + cw]
                if c < nch - 1:
                    nc.vector.tensor_copy(dst, ps[:])
                elif si % 4 == 0:
                    # diag boundary in cols [0,128); cols [128,256) all masked
                    nc.vector.tensor_add(dst[:, 0:P], ps[:, 0:P], mask128[:])
                    nc.vector.memset(dst[:, P:cw], NEG)
                else:
                    nc.vector.tensor_copy(dst[:, :cw - P], ps[:, :cw - P])
                    nc.vector.tensor_add(dst[:, cw - P:cw],
                                         ps[:, cw - P:cw], mask128[:])
                cmax = st_pool.tile([P, 1], F32, name="cmax", tag="st")
                nc.vector.reduce_max(cmax[:], dst, axis=mybir.AxisListType.X)
                cmaxes.append(cmax)
            while len(cmaxes) > 1:
                nxt = []
                for a, b in zip(cmaxes[::2], cmaxes[1::2]):
                    m = st_pool.tile([P, 1], F32, name="cmax", tag="st")
                    nc.vector.tensor_max(m[:], a[:], b[:])
                    nxt.append(m)
                if len(cmaxes) % 2:
                    nxt.append(cmaxes[-1])
                cmaxes = nxt
            negm = st_pool.tile([P, 1], F32, name="negm", tag="st")
            nc.vector.tensor_scalar_mul(negm[:], cmaxes[0][:], -SCALE)
            state[si] = (scores, negm)

        def pass_b1_exp(si):
            """exp chunks + one xbar-transpose to attnT for strip si."""
            nch = (si * P + P + TCH - 1) // TCH
            ntb = si + 1                      # 128-wide t-blocks in play
            scores, negm = state.pop(si)
            attnT = at_pool.tile([P, NSTRIP, P], BF16, name="attnT")
            partials = []
            dw = max(2 * P, (si % 4 + 1) * P)
            expf = exp_pool.tile([P, S], BF16, name="expf")
            for c in range(nch):
                cw = dw if c == nch - 1 else TCH
                part = st_pool.tile([P, 1], F32, name="part", tag="st")
                nc.scalar.activation(expf[:, c * TCH:c * TCH + cw],
                                     scores[:, c * TCH:c * TCH + cw],
                                     mybir.ActivationFunctionType.Exp,
                                     bias=negm[:], scale=SCALE,
                                     accum_out=part[:])
                partials.append(part)
            # one xbar per strip: attn row block -> [t, s] layout
            xbar(attnT[:, :ntb, :], expf[:, :ntb * P])
            state[(si, 'av')] = (attnT, partials)

        def pass_b1_av(si):
            """attn @ value for strip si (PE) + rowsum + ctx copy."""
            ntb = si + 1
            attnT, partials = state.pop((si, 'av'))
            cps = av_ps.tile([P, 2, TCH], F32, name="avps", tag="avps")
            # ctx[s, i] += sum_t attn[s, t] * value[t, i]
            for tb in range(ntb):
                for ih in range(2):
                    nc.tensor.matmul(cps[:, ih, :],
                                     attnT[:, tb, :],
                                     val[:, tb, ih * TCH:(ih + 1) * TCH],
                                     start=(tb == 0),
                                     stop=(tb == ntb - 1))
            rowsum = st_pool.tile([P, 1], F32, name="rowsum", tag="st")
            if len(partials) == 1:
                nc.vector.tensor_copy(rowsum[:], partials[0][:])
            else:
                nc.vector.tensor_add(rowsum[:], partials[0][:], partials[1][:])
                for part in partials[2:]:
                    nc.vector.tensor_add(rowsum[:], rowsum[:], part[:])
            ctxn = ctxn_pool.tile([P, 2 * TCH], BF16, name="ctxn", tag="ctxn")
            nc.vector.tensor_copy(
                ctxn[:].rearrange("p (a b) -> p a b", a=2), cps[:])
            ctxsb = ctxsb_pool.tile([P, OC, P], BF16, name="ctxsb")
            xbar(ctxsb[:], ctxn[:])
            recip = st_pool.tile([P, 1], F32, name="recip", tag="st")
            nc.vector.reciprocal(recip[:], rowsum[:])
            state[(si, 'b2t')] = (ctxsb, recip)

        def pass_b2_tail(si):
            """V-projection, normalize, store."""
            s0 = si * P
            ctxsb, recip = state.pop((si, 'b2t'))
            vp = av_ps.tile([P, 2, TCH], F32, name="avps", tag="avps")
            for dc in range(2):
                for ic in range(OC):
                    nc.tensor.matmul(
                        vp[:, dc, :], ctxsb[:, ic, :],
                        wvT[:, ic, dc * TCH:(dc + 1) * TCH],
                        start=(ic == 0), stop=(ic == OC - 1))
                ob = ob_pool.tile([P, TCH], F32, name="ob")
                nc.scalar.activation(ob[:], vp[:, dc, :],
                                     mybir.ActivationFunctionType.Copy,
                                     scale=recip[:])
                nc.scalar.dma_start(
                    out_d.ap()[s0:s0 + P, dc * TCH:(dc + 1) * TCH], ob[:])

        loads(0)
        loads(1)
        loads(2)
        pass_a(0)
        pass_a(1)
        pass_b1_exp(0)
        pass_b1_av(0)
        for si in range(NSTRIP):
            if si + 3 < NSTRIP:
                loads(si + 3)
            if si + 1 < NSTRIP:
                pass_b1_exp(si + 1)
            if si in (2, 4, 6):
                m_chunk(si // 2)          # fill early-strip PE bubbles
            if si + 2 < NSTRIP:
                pass_a(si + 2)
            if si + 1 < NSTRIP:
                pass_b1_av(si + 1)
            pass_b2_tail(si)

    _mark(nc, 'end')
    nc.finalize()
    return nc


_NC_CACHE = None


def kernel(**inputs):
    global _NC_CACHE
    if _NC_CACHE is None:
        _NC_CACHE = build_nc()
    nc = _NC_CACHE
    query = np.ascontiguousarray(inputs["query"], dtype=np.float32)
    key = np.ascontiguousarray(inputs["key"], dtype=np.float32)
    value = np.ascontiguousarray(inputs["value"], dtype=np.float32)
    Wq = np.ascontiguousarray(inputs["Wq"], dtype=np.float32)
    Wk = np.ascontiguousarray(inputs["Wk"], dtype=np.float32)
    Wv = np.ascontiguousarray(inputs["Wv"], dtype=np.float32)
    in_maps = [
        {"query": query[i], "key": key[i], "value": value[i],
         "Wq": Wq, "Wk": Wk, "Wv": Wv}
        for i in range(N)
    ]
    res = run_bass_kernel_spmd(nc, in_maps, core_ids=list(range(N)))
    return np.stack([res.results[i]["out"] for i in range(N)], axis=0)


# revision 20
# speedup vs baseline: 1.2002x; 1.2002x over previous
"""Causal attention layer on 8 TRN2 NeuronCores, data-parallel over batch.

Per-core problem (batch element n = core id):
    q = query @ Wq.T ; k = key @ Wk.T              (f32r matmuls)
    scores[s,t] = q[s]·k[t]  for t <= s            (f32r)
    attn = softmax(32 * scores)  (the +1 additive mask cancels in softmax;
                                  -inf masking == skipping t > s)
    ctx[s,i] = sum_t attn[s,t] * value[t,i]        (bf16)
    out[s,:] = (ctx @ Wv.T) / rowsum               (bf16, normalization folded)

v2 layout: all bf16 transposes (attnT, ctx, WvT) ride the DMA XBAR
transpose engine (serialized on the SP HWDGE queue — concurrent xbar
transposes on both queues corrupt each other); only the f32 q/k strip
transposes stay on the PE.  The M = G @ key^T projection is interleaved
into the early strip iterations to fill the PE bubbles the softmax
chain leaves there.
"""
import numpy as np
from contextlib import ExitStack

import concourse.bass as bass
import concourse.tile as tile
from concourse import bacc, mybir
from concourse.bass_utils import run_bass_kernel_spmd
from concourse.masks import make_identity

F32 = mybir.dt.float32
F32R = mybir.dt.float32r
BF16 = mybir.dt.bfloat16

N, S, T, D = 8, 2048, 2048, 1024
P = 128
NSTRIP = S // P          # 16 query strips
TCH = 512                # t-chunk for score matmuls
CW = 512                 # projection chunk width
OC = D // P              # 8 chunks of the projection/feature dim
NCHK = T // CW           # 4 key chunks
SCALE = float(np.sqrt(np.float32(D)))  # 32.0
NEG = -1.0e30

QK_DT = F32R             # q/k projections + scores matmuls

PHASE_MARKS = []


def _mark(nc, label):
    PHASE_MARKS.append((label, nc.next_id()))


def _mm(nc, out, lhsT, rhs, dt, **kw):
    nc.tensor.matmul(out, lhsT.bitcast(dt), rhs.bitcast(dt), **kw)


def build_nc():
    PHASE_MARKS.clear()
    nc = bacc.Bacc("TRN2", target_bir_lowering=False, debug=False,
                   enable_asserts=False)
    _dma_rr = [0]

    def dma(out, in_):
        # startup loads/stores alternate the two HWDGE queues
        _dma_rr[0] ^= 1
        eng = nc.sync if _dma_rr[0] else nc.scalar
        return eng.dma_start(out, in_)

    def xbar(out, in_):
        # ALL xbar transposes on the SP queue (they corrupt if concurrent)
        return nc.sync.dma_start(out, in_, transpose=True)

    q_d = nc.dram_tensor("query", [S, D], F32, kind="ExternalInput")
    k_d = nc.dram_tensor("key", [T, D], F32, kind="ExternalInput")
    v_d = nc.dram_tensor("value", [T, D], F32, kind="ExternalInput")
    wq_d = nc.dram_tensor("Wq", [D, D], F32, kind="ExternalInput")
    wk_d = nc.dram_tensor("Wk", [D, D], F32, kind="ExternalInput")
    wv_d = nc.dram_tensor("Wv", [D, D], F32, kind="ExternalInput")
    out_d = nc.dram_tensor("out", [S, D], F32, kind="ExternalOutput")

    with tile.TileContext(nc) as tc, ExitStack() as ctx:
        const = ctx.enter_context(tc.tile_pool(name="const", bufs=1))
        wt_pool = ctx.enter_context(tc.tile_pool(name="wt", bufs=1))
        wvt_pool = ctx.enter_context(tc.tile_pool(name="wvt", bufs=1))
        kt_pool = ctx.enter_context(tc.tile_pool(name="kt", bufs=1))
        val_pool = ctx.enter_context(tc.tile_pool(name="val", bufs=1))
        stage = ctx.enter_context(tc.tile_pool(name="stage", bufs=2))
        qst_pool = ctx.enter_context(tc.tile_pool(name="qst", bufs=1))
        int_pool = ctx.enter_context(tc.tile_pool(name="inT", bufs=1))
        qts_pool = ctx.enter_context(tc.tile_pool(name="qts", bufs=1))
        sc_pool = ctx.enter_context(tc.tile_pool(name="scores", bufs=2))
        exp_pool = ctx.enter_context(tc.tile_pool(name="exp", bufs=1))
        at_pool = ctx.enter_context(tc.tile_pool(name="attnT", bufs=1))
        ctxn_pool = ctx.enter_context(tc.tile_pool(name="ctxn", bufs=1))
        ctxsb_pool = ctx.enter_context(tc.tile_pool(name="ctxsb", bufs=1))
        ob_pool = ctx.enter_context(tc.tile_pool(name="outb", bufs=1))
        st_pool = ctx.enter_context(tc.tile_pool(name="stats", bufs=32))
        sc_ps = ctx.enter_context(tc.tile_pool(name="scps", bufs=2, space="PSUM"))
        tp_ps = ctx.enter_context(tc.tile_pool(name="tpps", bufs=2, space="PSUM"))
        av_ps = ctx.enter_context(tc.tile_pool(name="avps", bufs=2, space="PSUM"))

        ident = const.tile([P, P], F32)
        make_identity(nc, ident)
        # mask128[p, x] = 0 if x <= p else -1e30 (universal diag boundary)
        mask128 = const.tile([P, P], BF16)
        nc.gpsimd.memset(mask128[:], 0.0)
        nc.gpsimd.affine_select(
            out=mask128[:], in_=mask128[:],
            compare_op=mybir.AluOpType.is_ge, fill=NEG,
            base=0, channel_multiplier=1, pattern=[[-1, P]])

        _mm_ps_rr = [0]

        def mm_ps_tile():
            # rotate [P, TCH] psum tiles across the two 2-buf pools
            _mm_ps_rr[0] ^= 1
            pool = sc_ps if _mm_ps_rr[0] else tp_ps
            return pool.tile([P, TCH], F32, name="mmps",
                             tag="sc" if _mm_ps_rr[0] else "tp")

        # ---- phase G: gT[j, i] = (Wq^T @ Wk)^T = Wk^T @ Wq ----
        _mark(nc, 'phaseG')
        # prefetch key strips 0-1 for kT chunk 0 through the (still idle)
        # scores pool so the stage pool stays free for the weight stream
        xt0 = int_pool.tile([P, OC, CW], QK_DT, name="inT", tag="inT")
        k0stg = []
        for sl in range(2):
            stg = sc_pool.tile([P, D], F32, name="scores")
            nc.gpsimd.dma_start(stg[:], k_d.ap()[sl * P:(sl + 1) * P, :])
            k0stg.append(stg)

        def kT0_group(first):
            sls = (0, 1) if first else (2, 3)
            for sl in sls:
                stg = k0stg[sl]
                for g in range(0, OC, 4):
                    ps = tp_ps.tile([P, TCH], F32, name="mmps", tag="tp")
                    for c in range(4):
                        nc.tensor.transpose(
                            ps[:, c * P:(c + 1) * P],
                            stg[:, (g + c) * P:(g + c + 1) * P], ident[:])
                    nc.vector.tensor_copy(
                        xt0[:, g:g + 4, sl * P:(sl + 1) * P],
                        ps.rearrange("p (c s) -> p c s", c=4))
            if first:
                for sl in (2, 3):
                    stg = sc_pool.tile([P, D], F32, name="scores")
                    nc.gpsimd.dma_start(
                        stg[:], k_d.ap()[sl * P:(sl + 1) * P, :])
                    k0stg.append(stg)

        wqwk = kt_pool.tile([P, 16, D], QK_DT, name="wqwk", tag="kt")
        for r in range(OC):
            for w_dram, slot in ((wq_d, r), (wk_d, OC + r)):
                for h in range(2):
                    stg = stage.tile([P, TCH], F32, name="stgh", tag="stage")
                    dma(stg[:], w_dram.ap()[r * P:(r + 1) * P,
                                            h * TCH:(h + 1) * TCH])
                    nc.vector.tensor_copy(
                        wqwk[:, slot, h * TCH:(h + 1) * TCH], stg[:])
        gT = wt_pool.tile([P, OC, D], QK_DT, name="wT", tag="wT")
        # o-strip-streamed accumulation: 8 psum banks live at once so the
        # per-oc matmul work covers the per-oc DMA
        for half in range(2):
            grp = []
            for ih, jcg in ((half, 0), (1 - half, 1)):
                if jcg == 0:
                    accs = [mm_ps_tile()[:] for _ in range(4)]
                else:
                    acc_b = av_ps.tile([P, 2, TCH], F32, name="avps",
                                       tag="avps")
                    acc_c = av_ps.tile([P, 2, TCH], F32, name="avps",
                                       tag="avps")
                    accs = [acc_b[:, 0, :], acc_b[:, 1, :],
                            acc_c[:, 0, :], acc_c[:, 1, :]]
                grp.append((ih, jcg, accs))
            for oc in range(OC):
                for ih, jcg, accs in grp:
                    for j in range(4):
                        jc = jcg * 4 + j
                        _mm(nc, accs[j],
                            wqwk[:, OC + oc, jc * P:(jc + 1) * P],
                            wqwk[:, oc, ih * TCH:(ih + 1) * TCH], QK_DT,
                            start=(oc == 0), stop=(oc == OC - 1))
            for ih, jcg, accs in grp:
                for j in range(4):
                    jc = jcg * 4 + j
                    nc.vector.tensor_copy(
                        gT[:, jc, ih * TCH:(ih + 1) * TCH], accs[j])
            kT0_group(first=(half == 0))

        # ---- WvT (bf16) via xbar: wvT[i_loc, ic, j] = Wv[j, ic*128+i_loc]
        _mark(nc, 'phaseWv')
        wvT = wvt_pool.tile([P, OC, D], BF16, name="wvT")
        for js in range(OC):
            stg = stage.tile([P, D], F32, name="stage", tag="stage")
            dma(stg[:], wv_d.ap()[js * P:(js + 1) * P, :])
            wvb = ctxn_pool.tile([P, D], BF16, name="ctxn", tag="ctxn")
            nc.scalar.activation(wvb[:], stg[:],
                                 mybir.ActivationFunctionType.Copy)
            xbar(wvT[:, :, js * P:(js + 1) * P], wvb[:])

        def in_transpose_chunk(x_dram, c0):
            """x[c0:c0+CW, :D] -> xT tile [128(i_loc), OC, CW] (PE)."""
            xt = int_pool.tile([P, OC, CW], QK_DT, name="inT", tag="inT")
            for sl in range(CW // P):
                stg = stage.tile([P, D], F32, name="stage", tag="stage")
                nc.gpsimd.dma_start(
                    stg[:], x_dram.ap()[c0 + sl * P:c0 + (sl + 1) * P, :])
                for g in range(0, OC, 4):
                    ps = tp_ps.tile([P, TCH], F32, name="mmps", tag="tp")
                    for c in range(4):
                        nc.tensor.transpose(
                            ps[:, c * P:(c + 1) * P],
                            stg[:, (g + c) * P:(g + c + 1) * P], ident[:])
                    nc.vector.tensor_copy(
                        xt[:, g:g + 4, sl * P:(sl + 1) * P],
                        ps.rearrange("p (c s) -> p c s", c=4))
            return xt

        # kt written chunk-by-chunk; chunks 1-3 are emitted inside the
        # strip loop so the M matmuls fill early-strip PE bubbles
        kt = kt_pool.tile([P, OC, T], QK_DT, name="kt", tag="kt")

        def m_chunk(sc, kT_in=None):
            """M[:, sc*CW:(sc+1)*CW] = G @ key[sc-chunk]^T."""
            _mark(nc, f'M{sc}')
            if kT_in is None:
                kT_in = in_transpose_chunk(k_d, sc * CW)
            for ic in range(OC):
                ps = mm_ps_tile()
                for jc in range(OC):
                    _mm(nc, ps[:, :CW], gT[:, jc, ic * P:(ic + 1) * P],
                        kT_in[:, jc, :], QK_DT,
                        start=(jc == 0), stop=(jc == OC - 1))
                # ACT copy keeps the DVE queue free for the softmax chain
                nc.scalar.activation(
                    kt[:, ic, sc * CW:(sc + 1) * CW], ps[:, :CW],
                    mybir.ActivationFunctionType.Copy)

        m_chunk(0, kT_in=xt0)

        _mark(nc, 'phaseV')
        val = val_pool.tile([P, T // P, D], BF16)
        state = {}

        def loads(si):
            """value blocks 2si,2si+1 + query strip si (prefetch)."""
            vstgs = []
            for tb in (2 * si, 2 * si + 1):
                if tb >= T // P:
                    continue
                vstg = stage.tile([P, D], F32, name="stage", tag="stage")
                nc.gpsimd.dma_start(vstg[:],
                                    v_d.ap()[tb * P:(tb + 1) * P, :])
                vstgs.append((tb, vstg))
            state[(si, 'v')] = vstgs
            stg = qst_pool.tile([P, D], F32, name="qst")
            nc.scalar.dma_start(stg[:], q_d.ap()[si * P:(si + 1) * P, :])
            state[(si, 'q')] = stg

        def val_casts(si):
            """DVE casts stage -> val bf16; emitted at the END of the same
            iteration the DMAs were issued in, so the DVE never waits."""
            for tb, vstg in state.pop((si, 'v'), []):
                nc.vector.tensor_copy(val[:, tb, :], vstg[:])

        def pass_a(si):
            """scores chunks + row max for strip si (PE + DVE)."""
            _mark(nc, f'strip{si}')
            s0 = si * P
            nch = (s0 + P + TCH - 1) // TCH   # t-chunks covering [0, s0+128)
            stg = state.pop((si, 'q'))
            qts = qts_pool.tile([P, OC, P], QK_DT, name="qts")
            for g in range(0, OC, 4):
                ps = tp_ps.tile([P, TCH], F32, name="mmps", tag="tp")
                for c in range(4):
                    nc.tensor.transpose(
                        ps[:, c * P:(c + 1) * P],
                        stg[:, (g + c) * P:(g + c + 1) * P], ident[:])
                nc.vector.tensor_copy(
                    qts[:, g:g + 4, :],
                    ps.rearrange("p (c s) -> p c s", c=4))

            scores = sc_pool.tile([P, S], F32, name="scores")
            # diag chunk first so its (mask + max) tail overlaps the other
            # chunks' matmuls (f32r needs moving dim >= 256)
            dw = max(2 * P, (si % 4 + 1) * P)
            cmaxes = []
            for c in [nch - 1] + list(range(nch - 1)):
                cw = dw if c == nch - 1 else TCH
                ps = sc_ps.tile([P, TCH], F32, name="mmps", tag="sc")
                for oc in range(OC):
                    _mm(nc, ps[:, :cw], qts[:, oc, :],
                        kt[:, oc, c * TCH:c * TCH + cw], QK_DT,
                        start=(oc == 0), stop=(oc == OC - 1))
                dst = scores[:, c * TCH:c * TCH + cw]
                if c < nch - 1:
                    nc.vector.tensor_copy(dst, ps[:])
                elif si % 4 == 0:
                    # diag boundary in cols [0,128); cols [128,256) all masked
                    nc.vector.tensor_add(dst[:, 0:P], ps[:, 0:P], mask128[:])
                    nc.vector.memset(dst[:, P:cw], NEG)
                else:
                    nc.vector.tensor_copy(dst[:, :cw - P], ps[:, :cw - P])
                    nc.vector.tensor_add(dst[:, cw - P:cw],
                                         ps[:, cw - P:cw], mask128[:])
                cmax = st_pool.tile([P, 1], F32, name="cmax", tag="st")
                nc.vector.reduce_max(cmax[:], dst, axis=mybir.AxisListType.X)
                cmaxes.append(cmax)
            while len(cmaxes) > 1:
                nxt = []
                for a, b in zip(cmaxes[::2], cmaxes[1::2]):
                    m = st_pool.tile([P, 1], F32, name="cmax", tag="st")
                    nc.vector.tensor_max(m[:], a[:], b[:])
                    nxt.append(m)
                if len(cmaxes) % 2:
                    nxt.append(cmaxes[-1])
                cmaxes = nxt
            negm = st_pool.tile([P, 1], F32, name="negm", tag="st")
            nc.vector.tensor_scalar_mul(negm[:], cmaxes[0][:], -SCALE)
            state[si] = (scores, negm)

        def pass_b1_exp(si):
            """exp chunks + one xbar-transpose to attnT for strip si."""
            nch = (si * P + P + TCH - 1) // TCH
            ntb = si + 1                      # 128-wide t-blocks in play
            scores, negm = state.pop(si)
            attnT = at_pool.tile([P, NSTRIP, P], BF16, name="attnT")
            partials = []
            dw = max(2 * P, (si % 4 + 1) * P)
            expf = exp_pool.tile([P, S], BF16, name="expf")
            for c in range(nch):
                cw = dw if c == nch - 1 else TCH
                part = st_pool.tile([P, 1], F32, name="part", tag="st")
                nc.scalar.activation(expf[:, c * TCH:c * TCH + cw],
                                     scores[:, c * TCH:c * TCH + cw],
                                     mybir.ActivationFunctionType.Exp,
                                     bias=negm[:], scale=SCALE,
                                     accum_out=part[:])
                partials.append(part)
            # one xbar per strip: attn row block -> [t, s] layout
            xbar(attnT[:, :ntb, :], expf[:, :ntb * P])
            state[(si, 'av')] = (attnT, partials)

        def pass_b1_av(si):
            """attn @ value for strip si (PE) + rowsum + ctx copy."""
            ntb = si + 1
            attnT, partials = state.pop((si, 'av'))
            cps = av_ps.tile([P, 2, TCH], F32, name="avps", tag="avps")
            # ctx[s, i] += sum_t attn[s, t] * value[t, i]
            for tb in range(ntb):
                for ih in range(2):
                    nc.tensor.matmul(cps[:, ih, :],
                                     attnT[:, tb, :],
                                     val[:, tb, ih * TCH:(ih + 1) * TCH],
                                     start=(tb == 0),
                                     stop=(tb == ntb - 1))
            rowsum = st_pool.tile([P, 1], F32, name="rowsum", tag="st")
            if len(partials) == 1:
                nc.vector.tensor_copy(rowsum[:], partials[0][:])
            else:
                nc.vector.tensor_add(rowsum[:], partials[0][:], partials[1][:])
                for part in partials[2:]:
                    nc.vector.tensor_add(rowsum[:], rowsum[:], part[:])
            ctxn = ctxn_pool.tile([P, 2 * TCH], BF16, name="ctxn", tag="ctxn")
            nc.vector.tensor_copy(
                ctxn[:].rearrange("p (a b) -> p a b", a=2), cps[:])
            ctxsb = ctxsb_pool.tile([P, OC, P], BF16, name="ctxsb")
            xbar(ctxsb[:], ctxn[:])
            recip = st_pool.tile([P, 1], F32, name="recip", tag="st")
            nc.vector.reciprocal(recip[:], rowsum[:])
            state[(si, 'b2t')] = (ctxsb, recip)

        def pass_b2_tail(si):
            """V-projection, normalize, store."""
            s0 = si * P
            ctxsb, recip = state.pop((si, 'b2t'))
            vp = av_ps.tile([P, 2, TCH], F32, name="avps", tag="avps")
            for dc in range(2):
                for ic in range(OC):
                    nc.tensor.matmul(
                        vp[:, dc, :], ctxsb[:, ic, :],
                        wvT[:, ic, dc * TCH:(dc + 1) * TCH],
                        start=(ic == 0), stop=(ic == OC - 1))
                ob = ob_pool.tile([P, TCH], F32, name="ob")
                nc.scalar.activation(ob[:], vp[:, dc, :],
                                     mybir.ActivationFunctionType.Copy,
                                     scale=recip[:])
                nc.scalar.dma_start(
                    out_d.ap()[s0:s0 + P, dc * TCH:(dc + 1) * TCH], ob[:])

        loads(0)
        loads(1)
        loads(2)
        val_casts(0)
        pass_a(0)
        pass_a(1)
        pass_b1_exp(0)
        pass_b1_av(0)
        val_casts(1)
        val_casts(2)
        for si in range(NSTRIP):
            if si + 3 < NSTRIP:
                loads(si + 3)
            if si + 1 < NSTRIP:
                pass_b1_exp(si + 1)
            if si in (2, 4, 6):
                m_chunk(si // 2)          # fill early-strip PE bubbles
            if si + 2 < NSTRIP:
                pass_a(si + 2)
            if si + 1 < NSTRIP:
                pass_b1_av(si + 1)
            pass_b2_tail(si)
            val_casts(si + 3)

    _mark(nc, 'end')
    nc.finalize()
    return nc


_NC_CACHE = None


def kernel(**inputs):
    global _NC_CACHE
    if _NC_CACHE is None:
        _NC_CACHE = build_nc()
    nc = _NC_CACHE
    query = np.ascontiguousarray(inputs["query"], dtype=np.float32)
    key = np.ascontiguousarray(inputs["key"], dtype=np.float32)
    value = np.ascontiguousarray(inputs["value"], dtype=np.float32)
    Wq = np.ascontiguousarray(inputs["Wq"], dtype=np.float32)
    Wk = np.ascontiguousarray(inputs["Wk"], dtype=np.float32)
    Wv = np.ascontiguousarray(inputs["Wv"], dtype=np.float32)
    in_maps = [
        {"query": query[i], "key": key[i], "value": value[i],
         "Wq": Wq, "Wk": Wk, "Wv": Wv}
        for i in range(N)
    ]
    res = run_bass_kernel_spmd(nc, in_maps, core_ids=list(range(N)))
    return np.stack([res.results[i]["out"] for i in range(N)], axis=0)
